# revision 5
# baseline (speedup 1.0000x reference)
"""Trainium2 Bass kernel for complex depthwise batchnorm (training-mode stats).

v3 design, 8 NeuronCores, batch N split across cores, transposed layout:
each core's shard [NS=2048, D=2056] is cast to bf16 and transposed host-side
to [DP=2176, NS] so (c,f) dims live on SBUF partitions (17 chunks of 128) and
batch is the free axis. Pure streaming design (no PE/PSUM round-trip: PSUM
evacuation costs as much as direct streaming, measured).

Per-chunk streams, balanced across DVE/ACT/Pool by per-chunk sets:
  Phase A: xr stats via DVE bn_stats (mean+var in one pass), xi stats via
    DVE bn_stats or ACT Square/Identity+accum (STATS_ACT chunks); cross
    product on Pool, summed by ACT Identity+accum.
  Phase B: t2 = Z*x + bias' via DVE/ACT/Pool tensor_scalar (T2_* sets);
    y = (x*Z) + t2 via DVE scalar_tensor_tensor (2x perf mode).

AllReduce of raw sums [sum_r, sum_i, ssq_r, ssq_i, cross] per (c,f), in 4
chunk-groups (3,5,5,4); a warmup collective at t=0 absorbs the ~25us
first-collective barrier skew while the first loads stream. Staging DMAs
ride the ACT queue, result gathers the sync queue, so bulk loads never
block behind collective waits. bf16 end-to-end: rel err ~3e-3 vs 2e-2 gate.
"""

import numpy as np
import ml_dtypes

N, C, F = 16384, 8, 257
D = C * F            # 2056
P = 128
NCH = 17             # ceil(D / 128)
DP = NCH * P         # 2176 (zero-padded tail rows)
N_CORES = 8
NS = N // N_CORES    # 2048 (free dim per core)
SUB = 512            # bn_stats hardware window
NSUB = NS // SUB     # 4
EPS = 1e-6
DELTA_MAX = 1e8
INV_N = 1.0 / N

GROUPS = [[0, 1, 2], [3, 4, 5, 6, 7], [8, 9, 10, 11, 12], [13, 14, 15, 16]]
NQ = 5               # stat quantities per chunk: sr, si, ssr, ssi, cross

# xi-stats on ACT (Square+accum / Identity+accum) instead of DVE bn_stats
STATS_ACT = {1, 3, 5, 7, 9, 11, 13, 15}
# phase-B t2 pair engine per chunk: ACT default, POOL/DVE for these
T2_POOL = {2, 6, 10, 14, 16}
T2_DVE = {0, 8}

_CACHE = {}


def _build():
    import concourse.bacc as bacc
    import concourse.tile as tile
    import concourse.mybir as mybir

    f32 = mybir.dt.float32
    bf16 = mybir.dt.bfloat16
    Alu = mybir.AluOpType
    Act = mybir.ActivationFunctionType

    nc = bacc.Bacc("TRN2", target_bir_lowering=False, debug=False,
                   num_devices=N_CORES)

    xrt = nc.dram_tensor("xrt", [DP, NS], bf16, kind="ExternalInput").ap()
    xit = nc.dram_tensor("xit", [DP, NS], bf16, kind="ExternalInput").ap()
    # wp columns: 5 quantities x 17 chunk-cols, q-major (q*NCH + c)
    wp = nc.dram_tensor("wp", [P, 5 * NCH], f32, kind="ExternalInput").ap()
    yrt = nc.dram_tensor("yrt", [DP, NS], bf16, kind="ExternalOutput").ap()
    yit = nc.dram_tensor("yit", [DP, NS], bf16, kind="ExternalOutput").ap()

    with tile.TileContext(nc) as tc:
        with (
            tc.tile_pool(name="keep", bufs=1) as keep,
            tc.tile_pool(name="bs", bufs=4) as bsp,
            tc.tile_pool(name="crp", bufs=2) as crp,
            tc.tile_pool(name="t2p", bufs=2) as t2p,
            tc.tile_pool(name="yo", bufs=2) as yop,
            tc.tile_pool(name="co", bufs=6) as cop,
            tc.tile_pool(name="dram", bufs=1, space="DRAM") as dram,
        ):
            V = nc.vector
            S = nc.scalar
            G = nc.gpsimd
            SY = nc.sync

            wpt = keep.tile([P, 5 * NCH], f32, name="wpt")
            SY.dma_start(out=wpt[:], in_=wp[:, :])

            # garbage-output tile for ACT accum ops
            dump_a = keep.tile([P, NS], bf16, name="dump_a")

            # warmup collective: absorbs the first-collective barrier
            # rendezvous while the input loads stream in
            wu_in = dram.tile([P, 1], f32, name="wu_in")
            wu_out = dram.tile([P, 1], f32, name="wu_out",
                               addr_space="Shared")
            wu_sb = keep.tile([P, 1], f32, name="wu_sb")
            V.memset(wu_sb[:], 0.0)
            S.dma_start(out=wu_in[:, :], in_=wu_sb[:])
            G.collective_compute(
                "AllReduce", Alu.add,
                replica_groups=[list(range(N_CORES))],
                ins=[wu_in[:].opt()], outs=[wu_out[:].opt()])

            # ---------------- all input loads up front (sync queue) -----
            xr_c, xi_c = [None] * NCH, [None] * NCH
            for g in GROUPS:
                for c in g:
                    xt = keep.tile([P, NS], bf16, name=f"xr{c}")
                    SY.dma_start(out=xt[:], in_=xrt[c * P:(c + 1) * P, :])
                    yt = keep.tile([P, NS], bf16, name=f"xi{c}")
                    SY.dma_start(out=yt[:], in_=xit[c * P:(c + 1) * P, :])
                    xr_c[c] = xt
                    xi_c[c] = yt

            # per-group staging/result tiles
            cc_sb, mv_r, mv_i, gts = [], [], [], []
            cc_in, cc_out = [], []
            for gi, g in enumerate(GROUPS):
                ng = len(g)
                cc_sb.append(keep.tile([P, NQ * ng], f32, name=f"ccsb{gi}"))
                mv_r.append(keep.tile([P, 2 * ng], f32, name=f"mvr{gi}"))
                mv_i.append(keep.tile([P, 2 * ng], f32, name=f"mvi{gi}"))
                gts.append(keep.tile([P, NQ * ng], f32, name=f"gt{gi}"))
                cc_in.append(dram.tile([P, NQ * ng], f32, name=f"ccin{gi}"))
                cc_out.append(dram.tile([P, NQ * ng], f32, name=f"ccout{gi}",
                                        addr_space="Shared"))

            # coefficient tiles, one column per chunk
            zrr = keep.tile([P, NCH], f32, name="zrr")
            zri = keep.tile([P, NCH], f32, name="zri")
            zir = keep.tile([P, NCH], f32, name="zir")
            zii = keep.tile([P, NCH], f32, name="zii")
            brp = keep.tile([P, NCH], f32, name="brp")
            bip = keep.tile([P, NCH], f32, name="bip")

            def bn_tensor(xt, mv, j, ng, tag, c):
                """DVE bn_stats x4 + aggregate -> mv[:, j::ng] = [mean, var]"""
                bs = bsp.tile([P, NSUB, 6], f32, tag=tag, name=f"{tag}{c}")
                for s in range(NSUB):
                    V.bn_stats(out=bs[:, s, :],
                               in_=xt[:, s * SUB:(s + 1) * SUB])
                V.bn_aggr(out=mv[:, j::ng], in_=bs[:])

            def phase_a_chunk(gi, j, c):
                ng = len(GROUPS[gi])
                xt, yt = xr_c[c], xi_c[c]
                cs = cc_sb[gi]
                bn_tensor(xt, mv_r[gi], j, ng, "bsr", c)
                if c in STATS_ACT:
                    # raw sums straight into the collective payload
                    S.activation(dump_a[:], yt[:], Act.Identity,
                                 accum_out=cs[:, ng + j:ng + j + 1])
                    S.activation(dump_a[:], yt[:], Act.Square,
                                 accum_out=cs[:, 3 * ng + j:3 * ng + j + 1])
                else:
                    bn_tensor(yt, mv_i[gi], j, ng, "bsi", c)
                # cross product on Pool, summed on ACT
                cr = crp.tile([P, NS], bf16, tag="cr", name=f"cr{c}")
                G.tensor_tensor(cr[:], xt[:], yt[:], Alu.mult)
                S.activation(dump_a[:], cr[:], Act.Identity,
                             accum_out=cs[:, 4 * ng + j:4 * ng + j + 1])

            def prep_bn_sums(mv, cs, q0, q1, ng, gi, nm):
                """[mean,var] -> raw [sum, sumsq] into cc slots q0, q1.
                Writes only the columns of chunks that used bn_stats."""
                V.tensor_scalar_mul(cs[:, q0 * ng:(q0 + 1) * ng],
                                    mv[:, 0:ng], float(NS))
                tm = cop.tile([P, ng], f32, tag=f"tm{nm}", name=f"tm{nm}{gi}")
                V.tensor_tensor(tm[:], mv[:, 0:ng], mv[:, 0:ng], Alu.mult)
                V.tensor_tensor(tm[:], tm[:], mv[:, ng:2 * ng], Alu.add)
                V.tensor_scalar_mul(cs[:, q1 * ng:(q1 + 1) * ng],
                                    tm[:], float(NS))

            def stage_group(gi):
                g = GROUPS[gi]
                ng = len(g)
                cs = cc_sb[gi]
                prep_bn_sums(mv_r[gi], cs, 0, 2, ng, gi, "r")
                # xi bn chunks only: ACT chunks already wrote raw sums; their
                # mv_i columns are never written, so overwrite per-column
                for j, c in enumerate(g):
                    if c in STATS_ACT:
                        continue
                    V.tensor_scalar_mul(cs[:, ng + j:ng + j + 1],
                                        mv_i[gi][:, j:j + 1], float(NS))
                    tm = cop.tile([P, 1], f32, tag="tmi", name=f"tmi{c}")
                    V.tensor_tensor(tm[:], mv_i[gi][:, j:j + 1],
                                    mv_i[gi][:, j:j + 1], Alu.mult)
                    V.tensor_tensor(tm[:], tm[:],
                                    mv_i[gi][:, ng + j:ng + j + 1], Alu.add)
                    V.tensor_scalar_mul(cs[:, 3 * ng + j:3 * ng + j + 1],
                                        tm[:], float(NS))
                S.dma_start(out=cc_in[gi][:, :], in_=cs[:])
                G.collective_compute(
                    "AllReduce", Alu.add,
                    replica_groups=[list(range(N_CORES))],
                    ins=[cc_in[gi][:].opt()], outs=[cc_out[gi][:].opt()])
                SY.dma_start(out=gts[gi][:], in_=cc_out[gi][:, :])

            def coeff_math(gi):
                g = GROUPS[gi]
                ng = len(g)
                lo, hi = g[0], g[-1] + 1
                cs = slice(lo, hi)
                gt = gts[gi]

                def q(t, i):
                    return t[:, i * ng:(i + 1) * ng]

                def w(i):
                    return wpt[:, i * NCH + lo:i * NCH + hi]

                def stile(name):
                    return keep.tile([P, ng], f32, name=f"{name}_{lo}")

                mr = stile("mr")
                V.tensor_scalar_mul(mr[:], q(gt, 0), INV_N)
                mi = stile("mi")
                V.tensor_scalar_mul(mi[:], q(gt, 1), INV_N)

                mr2 = stile("mr2")
                V.tensor_tensor(mr2[:], mr[:], mr[:], Alu.mult)
                mi2 = stile("mi2")
                V.tensor_tensor(mi2[:], mi[:], mi[:], Alu.mult)
                mri = stile("mri")
                V.tensor_tensor(mri[:], mr[:], mi[:], Alu.mult)

                vrr = stile("vrr")
                V.scalar_tensor_tensor(vrr[:], q(gt, 2), INV_N, mr2[:],
                                       Alu.mult, Alu.subtract)
                vii = stile("vii")
                V.scalar_tensor_tensor(vii[:], q(gt, 3), INV_N, mi2[:],
                                       Alu.mult, Alu.subtract)
                vri = stile("vri")
                V.scalar_tensor_tensor(vri[:], q(gt, 4), INV_N, mri[:],
                                       Alu.mult, Alu.subtract)

                tau = stile("tau")
                V.tensor_tensor(tau[:], vrr[:], vii[:], Alu.add)
                dl = stile("dl")
                V.tensor_tensor(dl[:], vrr[:], vii[:], Alu.mult)
                vri2 = stile("vri2")
                V.tensor_tensor(vri2[:], vri[:], vri[:], Alu.mult)
                delta = stile("delta")
                V.tensor_tensor(delta[:], dl[:], vri2[:], Alu.subtract)
                V.tensor_scalar(delta[:], delta[:], EPS, DELTA_MAX,
                                Alu.max, Alu.min)

                s_t = stile("s_t")
                S.activation(s_t[:], delta[:], Act.Sqrt)
                targ = stile("targ")
                V.scalar_tensor_tensor(targ[:], s_t[:], 2.0, tau[:],
                                       Alu.mult, Alu.add)
                t_t = stile("t_t")
                S.activation(t_t[:], targ[:], Act.Sqrt)
                stt_ = stile("stt")
                V.tensor_tensor(stt_[:], s_t[:], t_t[:], Alu.mult)
                rst = stile("rst")
                V.reciprocal(rst[:], stt_[:])

                a1 = stile("a1")
                V.tensor_tensor(a1[:], s_t[:], vii[:], Alu.add)
                urr = stile("urr")
                V.tensor_tensor(urr[:], a1[:], rst[:], Alu.mult)
                a2 = stile("a2")
                V.tensor_tensor(a2[:], s_t[:], vrr[:], Alu.add)
                uii = stile("uii")
                V.tensor_tensor(uii[:], a2[:], rst[:], Alu.mult)
                uri = stile("uri")
                V.scalar_tensor_tensor(uri[:], vri[:], -1.0, rst[:],
                                       Alu.mult, Alu.mult)

                def mix(zt, wa, ua, wb, ub, nm):
                    g1 = stile(nm + "g1")
                    V.tensor_tensor(g1[:], wa, ua[:], Alu.mult)
                    g2 = stile(nm + "g2")
                    V.tensor_tensor(g2[:], wb, ub[:], Alu.mult)
                    V.tensor_tensor(zt[:, cs], g1[:], g2[:], Alu.add)

                mix(zrr, w(0), urr, w(1), uri, "zrr")
                mix(zri, w(0), uri, w(1), uii, "zri")
                mix(zir, w(1), urr, w(2), uri, "zir")
                mix(zii, w(1), uri, w(2), uii, "zii")

                def bias(bt, b0, za, zb, nm):
                    h1 = stile(nm + "h1")
                    V.tensor_tensor(h1[:], za[:, cs], mr[:], Alu.mult)
                    h2 = stile(nm + "h2")
                    V.tensor_tensor(h2[:], zb[:, cs], mi[:], Alu.mult)
                    h3 = stile(nm + "h3")
                    V.tensor_tensor(h3[:], h1[:], h2[:], Alu.add)
                    V.tensor_tensor(bt[:, cs], b0, h3[:], Alu.subtract)

                bias(brp, w(3), zrr, zri, "brp")
                bias(bip, w(4), zir, zii, "bip")

            def phase_b_chunk(c):
                xt, yt = xr_c[c], xi_c[c]
                col = slice(c, c + 1)
                yro = yop.tile([P, NS], bf16, tag="yro", name=f"yro{c}")
                yio = yop.tile([P, NS], bf16, tag="yio", name=f"yio{c}")
                t2r = t2p.tile([P, NS], bf16, tag="t2r", name=f"t2r{c}")
                t2i = t2p.tile([P, NS], bf16, tag="t2i", name=f"t2i{c}")
                # t2r = Zri*xi + Br' ; t2i = Zir*xr + Bi'
                if c in T2_POOL:
                    G.tensor_scalar(t2r[:], yt[:], zri[:, col], brp[:, col],
                                    Alu.mult, Alu.add)
                    G.tensor_scalar(t2i[:], xt[:], zir[:, col], bip[:, col],
                                    Alu.mult, Alu.add)
                elif c in T2_DVE:
                    V.tensor_scalar(t2r[:], yt[:], zri[:, col], brp[:, col],
                                    Alu.mult, Alu.add)
                    V.tensor_scalar(t2i[:], xt[:], zir[:, col], bip[:, col],
                                    Alu.mult, Alu.add)
                else:
                    S.activation(t2r[:], yt[:], Act.Identity,
                                 bias=brp[:, col], scale=zri[:, col])
                    S.activation(t2i[:], xt[:], Act.Identity,
                                 bias=bip[:, col], scale=zir[:, col])
                # yr = (xr*Zrr) + t2r ; yi = (xi*Zii) + t2i   (DVE stt)
                V.scalar_tensor_tensor(yro[:], xt[:], zrr[:, col], t2r[:],
                                       Alu.mult, Alu.add)
                V.scalar_tensor_tensor(yio[:], yt[:], zii[:, col], t2i[:],
                                       Alu.mult, Alu.add)
                SY.dma_start(out=yrt[c * P:(c + 1) * P, :], in_=yro[:])
                SY.dma_start(out=yit[c * P:(c + 1) * P, :], in_=yio[:])

            def finish_group(gi):
                coeff_math(gi)
                for c in GROUPS[gi]:
                    phase_b_chunk(c)

            # ---------------- schedule ----------------
            for gi, g in enumerate(GROUPS):
                for j, c in enumerate(g):
                    phase_a_chunk(gi, j, c)
                stage_group(gi)
                if gi >= 1:
                    finish_group(gi - 1)
            finish_group(len(GROUPS) - 2)
            finish_group(len(GROUPS) - 1)

    nc.compile()
    return nc


def get_nc():
    if "nc" not in _CACHE:
        _CACHE["nc"] = _build()
    return _CACHE["nc"]


def make_in_maps(xr, xi, Wrr, Wri, Wii, Br, Bi):
    bf = ml_dtypes.bfloat16
    xr2 = np.asarray(xr).reshape(N, D)
    xi2 = np.asarray(xi).reshape(N, D)
    xr_bf = xr2.astype(bf)
    xi_bf = xi2.astype(bf)

    # params -> [P, 5*NCH], q-major (q*NCH + chunk)
    def to_cols(a):
        v = np.zeros(DP, dtype=np.float32)
        v[:D] = np.asarray(a).reshape(D)
        return v.reshape(NCH, P).T          # [P, NCH], col c = chunk c

    wp = np.concatenate(
        [to_cols(a) for a in (Wrr, Wri, Wii, Br, Bi)], axis=1
    ).astype(np.float32)

    in_maps = []
    for r in range(N_CORES):
        xrt = np.zeros((DP, NS), dtype=bf)
        xrt[:D] = xr_bf[r * NS:(r + 1) * NS].T
        xit = np.zeros((DP, NS), dtype=bf)
        xit[:D] = xi_bf[r * NS:(r + 1) * NS].T
        in_maps.append({"xrt": xrt, "xit": xit, "wp": wp})
    return in_maps


def kernel(xr, xi, Wrr, Wri, Wii, Br, Bi):
    from concourse import bass_utils

    nc = get_nc()
    in_maps = make_in_maps(xr, xi, Wrr, Wri, Wii, Br, Bi)
    res = bass_utils.run_bass_kernel_spmd(nc, in_maps,
                                          core_ids=list(range(N_CORES)))
    yr = np.concatenate(
        [np.asarray(res.results[r]["yrt"])[:D].T for r in range(N_CORES)],
        axis=0).astype(np.float32)
    yi = np.concatenate(
        [np.asarray(res.results[r]["yit"])[:D].T for r in range(N_CORES)],
        axis=0).astype(np.float32)
    return yr.reshape(N, C, F), yi.reshape(N, C, F)
